# revision 24
# baseline (speedup 1.0000x reference)
"""Trainium2 Bass kernel for the Arcface loss forward.

Math (from the reference):
  xn = x / ||x||_F                     (global frobenius norm over the whole tensor)
  for each unordered pair (i<j) of the S axis:
      scores[b,(i,j)] = 5 * xn[b,i] @ xn[b,j].T          # [W, W]
      tgt[b,(i,j),w]  = first v with target[b,j,v] == target[b,i,w], else 0

Outputs: scores [B*P*W, W] f32, tgt [B*P*W] int32  (P = S*(S-1)/2 = 120)

Strategy: data-parallel over B across 8 cores (4 batches per core).
 - scale = sqrt(5)/||x|| folded into both GEMM operands (product carries
   5/||x||^2); computed on the host (cheap global reduction).
 - x[b,s] tiles are PE-transposed ([W,h] -> [h,W]) so the contraction dim h
   sits on partitions; compute dtype bf16 (fp32 accumulate in PSUM).
 - scores are written to DRAM as bf16 and upcast on the host (halves the
   dominant DMA traffic; adds ~1e-3 rel err, far below the 2e-2 gate).
 - tgt via exact small matmuls: a first-match-index table per (b,s) row is
   built with compares + max-reduce; gathered through one-hot matmuls.
 - pipeline: per-batch GEMM stream with next batch's transposes interleaved;
   2-bank PSUM drain tiles; drains split ScalarE/VectorE.
"""

import math

import numpy as np

import concourse.mybir as mybir
import concourse.tile as tile
from concourse import bacc
from concourse.bass_utils import run_bass_kernel_spmd
from concourse.masks import make_identity

# problem shape (hardcoded per harness contract)
B, S, W, H = 32, 16, 128, 256
P = S * (S - 1) // 2  # 120
N_CORES = 8
BL = B // N_CORES  # 4 batches per core
ROWS = BL * P * W  # 61440 output rows per core
NSH = 2  # h halves of 128
R = BL * S  # 64 target rows per core
SC = 4  # s-chunking of loads/transposes

FP = mybir.dt.float32
BF = mybir.dt.bfloat16
I32 = mybir.dt.int32
ALU = mybir.AluOpType
AF = mybir.ActivationFunctionType


def _pbase(i):
    # first pair index with first element i in row-major triu order
    return sum(S - 1 - k for k in range(i))


def build_nc():
    nc = bacc.Bacc("TRN2", target_bir_lowering=False, debug=False)

    x_d = nc.dram_tensor("x", [BL, S, W, H], FP, kind="ExternalInput")
    t_d = nc.dram_tensor("target", [BL, S, W], I32, kind="ExternalInput")
    sc_d = nc.dram_tensor("scale", [1, 1], FP, kind="ExternalInput")
    scores_d = nc.dram_tensor("scores", [ROWS, W], BF, kind="ExternalOutput")
    tgt_d = nc.dram_tensor("tgt", [ROWS], I32, kind="ExternalOutput")

    with tile.TileContext(nc) as tc:
        with (
            tc.tile_pool(name="consts", bufs=1) as consts,
            tc.tile_pool(name="xraw", bufs=BL) as xraw_pool,
            tc.tile_pool(name="xt", bufs=BL * NSH) as xt_pool,
            tc.tile_pool(name="outs", bufs=6) as out_pool,
            tc.tile_pool(name="small", bufs=2) as small,
            tc.tile_pool(name="intp", bufs=2) as intp,
            tc.tile_pool(name="cmps", bufs=16) as cmps,
            tc.tile_pool(name="ps_mm", bufs=3, space="PSUM") as ps_mm,
            tc.tile_pool(name="ps_tp", bufs=2, space="PSUM") as ps_tp,
        ):
            # ---------------- constants ----------------
            ident = consts.tile([128, 128], FP)
            make_identity(nc, ident[:])
            ident_bf = consts.tile([128, 128], BF)
            make_identity(nc, ident_bf[:])
            # wv[r, w] = W - w  (weight that makes max-reduce pick first match)
            wv = consts.tile([R, W], FP)
            nc.gpsimd.iota(
                wv[:], pattern=[[-1, W]], base=W, channel_multiplier=0,
                allow_small_or_imprecise_dtypes=True,
            )
            ones_row = consts.tile([1, 128], FP)
            nc.gpsimd.memset(ones_row[:], 1.0)

            # ---------------- PE warm-up (HAM clock gate) ----------------
            warm_in = consts.tile([128, 128], BF)
            nc.vector.memset(warm_in[:], 1.0)
            warm_ps = ps_mm.tile([128, 1024], FP, tag="ps_mm")
            NWARM = 90
            for k in range(NWARM):
                nc.tensor.matmul(
                    warm_ps[:, 0:128], warm_in[:], warm_in[:],
                    start=(k == 0), stop=(k == NWARM - 1),
                )

            # ---------------- input loads ----------------
            xraw = [[None] * (S // SC) for _ in range(BL)]
            for b in range(BL):
                qorder = reversed(range(S // SC)) if b == 0 else range(S // SC)
                for q in qorder:
                    xr = xraw_pool.tile([W, SC, H], FP, tag="xraw")
                    nc.sync.dma_start(
                        xr[:],
                        x_d[b, q * SC : (q + 1) * SC].rearrange("s w h -> w s h"),
                    )
                    xraw[b][q] = xr

            tmat_i = small.tile([R, W], I32)
            nc.sync.dma_start(tmat_i[:], t_d.ap().rearrange("b s w -> (b s) w"))
            tmat_f = small.tile([R, W], FP)
            nc.vector.tensor_copy(tmat_f[:], tmat_i[:])

            # host-computed norm scale -> broadcast to [128, 1]
            scale_sb = small.tile([1, 1], FP)
            nc.gpsimd.dma_start(scale_sb[:], sc_d[:, :])
            ps_bc = ps_tp.tile([128, 1], FP, tag="ps_tp")
            nc.tensor.matmul(
                ps_bc[:], ones_row[:], scale_sb[:], start=True, stop=True
            )
            scale128 = consts.tile([128, 1], FP)
            nc.vector.tensor_copy(scale128[:], ps_bc[:])

            # ---------------- int path: analysis (cheap, early) ----------
            # first-index table fi[(b,s), c] = min{w: target[b,s,w]==c} else 0
            maxv = intp.tile([R, 16], FP, tag="maxv")
            for c in range(16):
                eq = intp.tile([R, W], FP, tag="eq")
                nc.vector.tensor_scalar(
                    eq[:], tmat_f[:], float(c), None, op0=ALU.is_equal
                )
                val = intp.tile([R, W], FP, tag="val")
                nc.vector.tensor_tensor(val[:], eq[:], wv[:], op=ALU.mult)
                nc.vector.reduce_max(
                    maxv[:, c : c + 1], val[:], axis=mybir.AxisListType.X
                )
            gt0 = intp.tile([R, 16], FP, tag="gt0")
            nc.vector.tensor_scalar(gt0[:], maxv[:], 0.0, None, op0=ALU.is_gt)
            wmin = intp.tile([R, 16], FP, tag="wmin")
            nc.vector.tensor_scalar(
                wmin[:], maxv[:], -1.0, float(W), op0=ALU.mult, op1=ALU.add
            )
            fi = intp.tile([R, 16], FP, tag="fi")
            nc.vector.tensor_tensor(fi[:], gt0[:], wmin[:], op=ALU.mult)
            ps_fi = ps_tp.tile([16, R], FP, tag="ps_tp")
            nc.tensor.transpose(ps_fi[:], fi[:], ident[0:R, 0:R])
            fiT = intp.tile([16, R], BF, tag="fiT")
            nc.vector.tensor_copy(fiT[:], ps_fi[:])

            # tmatT[w, r] then one-hot OT[w, r, c] = (tmatT[w,r] == c)
            ps_tm = ps_tp.tile([128, R], FP, tag="ps_tp")
            nc.tensor.transpose(ps_tm[:], tmat_f[:], ident[0:R, 0:R])
            tmatT = intp.tile([128, R], FP, tag="tmatT")
            nc.vector.tensor_copy(tmatT[:], ps_tm[:])
            OT = intp.tile([128, R, 16], BF, tag="OT")
            for c in range(16):
                nc.vector.tensor_scalar(
                    OT[:, :, c], tmatT[:], float(c), None, op0=ALU.is_equal
                )

            # ---------------- pipeline building blocks ----------------
            xt = []
            for _b in range(BL):
                row = []
                for _h in range(NSH):
                    xt_t = xt_pool.tile([128, S * W], BF, tag="xt")
                    row.append(xt_t)
                xt.append(row)

            xs_cache = {}

            def emit_xs_unit(b, q, hf):
                key = (b, q)
                if key not in xs_cache:
                    xs = xraw_pool.tile([W, SC, H], BF, tag="xs")
                    nc.vector.tensor_scalar(
                        xs[:], xraw[b][q][:], scale128[:], None, op0=ALU.mult
                    )
                    xs_cache[key] = xs
                xs = xs_cache[key]
                ps_t = ps_tp.tile([128, SC * 128], BF, tag="ps_tp")
                for sl in range(SC):
                    nc.tensor.transpose(
                        ps_t[:, sl * 128 : (sl + 1) * 128],
                        xs[:, sl, hf * 128 : (hf + 1) * 128],
                        ident_bf[:],
                    )
                s0 = q * SC
                nc.vector.tensor_copy(
                    xt[b][hf][:, s0 * W : (s0 + SC) * W], ps_t[:]
                )

            def emit_xs_chunk(b, q):
                for hf in range(NSH):
                    emit_xs_unit(b, q, hf)

            drains = [0]

            def emit_gemm(b, i):
                j0 = i + 1
                L = S - 1 - i
                stage = out_pool.tile([128, L * W], BF, tag="outs")
                cols = L * W
                tiles = []  # (psum_tile, col0, ncols)
                c = 0
                while c < cols:
                    n = min(1024, cols - c)
                    pt = ps_mm.tile([128, 1024], FP, tag="ps_mm")
                    tiles.append((pt, c, n))
                    c += n
                for hf in range(NSH):
                    lhs = xt[b][hf][:, i * W : (i + 1) * W]
                    for pt, c0, n in tiles:
                        for g in range(0, n, 512):
                            m = min(512, n - g)
                            nc.tensor.matmul(
                                pt[:, g : g + m],
                                lhs,
                                xt[b][hf][
                                    :, j0 * W + c0 + g : j0 * W + c0 + g + m
                                ],
                                start=(hf == 0), stop=(hf == 1),
                            )
                dk = drains[0]
                for ti, (pt, c0, n) in enumerate(tiles):
                    dst = stage[:, c0 : c0 + n]
                    # two tiles of a group drain on different engines so the
                    # PSUM banks recycle in parallel; singles alternate
                    use_act = (ti == 0) if len(tiles) > 1 else (dk % 2 == 0)
                    if use_act:
                        nc.scalar.copy(dst, pt[:, :n])
                    else:
                        nc.vector.tensor_copy(dst, pt[:, :n])
                    dk += 1
                drains[0] = dk
                r0 = b * P + _pbase(i)
                nc.sync.dma_start(
                    scores_d.ap()[r0 * W : (r0 + L) * W, :].rearrange(
                        "(pp w) v -> w pp v", w=W
                    ),
                    stage[:].rearrange("w (pp v) -> w pp v", v=W),
                )

            cmp_sb = []
            tgt2d = tgt_d.ap().rearrange("(r v) -> r v", v=W)

            def emit_cmp_bank(kb):
                ps_cmp = ps_tp.tile([16, 512], BF, tag="ps_tp")
                for k in range(4):
                    rr = kb * 4 + k
                    nc.tensor.transpose(
                        ps_cmp[:, k * 128 : (k + 1) * 128],
                        OT[:, rr, :],
                        ident_bf[:],
                    )
                cs = cmps.tile([16, 512], BF, tag="cmp_sb")
                nc.vector.tensor_copy(cs[:], ps_cmp[:])
                cmp_sb.append(cs)

            def emit_tgt_group(bb, tt):
                # final matmul tgt[j, (r,w)] = fi[bb][t0[r,w], j]
                ps_tg = ps_tp.tile([16, 512], FP, tag="ps_tp")
                nc.tensor.matmul(
                    ps_tg[:],
                    fiT[:, bb * S : (bb + 1) * S],
                    cmp_sb[bb * 4 + tt][:],
                    start=True, stop=True,
                )
                tg_i = intp.tile([16, 512], I32, tag="tg_i")
                nc.vector.tensor_copy(tg_i[:], ps_tg[:])
                for k in range(4):
                    i = tt * 4 + k
                    if i >= S - 1:
                        continue
                    L = S - 1 - i
                    r0 = bb * P + _pbase(i)
                    nc.gpsimd.dma_start(
                        tgt2d[r0 : r0 + L, :],
                        tg_i[i + 1 : S, k * 128 : (k + 1) * 128],
                    )

            # ---------------- per-batch pipeline ----------------
            for q in reversed(range(S // SC)):
                emit_xs_chunk(0, q)
            for b in range(BL):
                i_order = (
                    list(reversed(range(S - 1))) if b == 0 else list(range(S - 1))
                )
                nxt = (
                    [(b + 1, q, hf) for q in range(S // SC) for hf in range(NSH)]
                    if b + 1 < BL
                    else []
                )
                for idx, i in enumerate(i_order):
                    emit_gemm(b, i)
                    if nxt and idx % 2 == 1:
                        emit_xs_unit(*nxt.pop(0))
                for u in nxt:
                    emit_xs_unit(*u)
                if b == 0:
                    for kb in range(R // 4):
                        emit_cmp_bank(kb)
                    for bb in range(BL):
                        for tt in range(4):
                            emit_tgt_group(bb, tt)

    nc.compile()
    return nc


_NC = None


def _get_nc():
    global _NC
    if _NC is None:
        _NC = build_nc()
    return _NC


def kernel(x: np.ndarray, target: np.ndarray):
    x = np.ascontiguousarray(x, dtype=np.float32)
    target = np.ascontiguousarray(target, dtype=np.int32)

    scale = np.float32(
        math.sqrt(5.0)
        / math.sqrt(float(np.sum(np.square(x, dtype=np.float64))))
    )
    sc = np.array([[scale]], dtype=np.float32)

    in_maps = []
    for c in range(N_CORES):
        in_maps.append(
            {
                "x": np.ascontiguousarray(x[c * BL : (c + 1) * BL]),
                "target": np.ascontiguousarray(target[c * BL : (c + 1) * BL]),
                "scale": sc,
            }
        )

    nc = _get_nc()
    res = run_bass_kernel_spmd(nc, in_maps, core_ids=list(range(N_CORES)))
    scores = np.concatenate(
        [np.asarray(r["scores"]) for r in res.results], axis=0
    ).astype(np.float32)
    tgt = np.concatenate([r["tgt"] for r in res.results], axis=0)
    return scores, tgt


# revision 28
# speedup vs baseline: 1.2221x; 1.2221x over previous
"""Trainium2 Bass kernel for the Arcface loss forward.

Math (from the reference):
  xn = x / ||x||_F                     (global frobenius norm over the whole tensor)
  for each unordered pair (i<j) of the S axis:
      scores[b,(i,j)] = 5 * xn[b,i] @ xn[b,j].T          # [W, W]
      tgt[b,(i,j),w]  = first v with target[b,j,v] == target[b,i,w], else 0

Outputs: scores [B*P*W, W] f32, tgt [B*P*W] int32  (P = S*(S-1)/2 = 120)

Strategy: data-parallel over B across 8 cores (4 batches per core).
 - scale = sqrt(5)/||x|| folded into both GEMM operands (product carries
   5/||x||^2); computed on the host (cheap global reduction).
 - x[b,s] tiles are PE-transposed ([W,h] -> [h,W]) so the contraction dim h
   sits on partitions; compute dtype bf16 (fp32 accumulate in PSUM).
 - scores are written to DRAM as bf16 and upcast on the host (halves the
   dominant DMA traffic; adds ~1e-3 rel err, far below the 2e-2 gate).
 - tgt via exact small matmuls: a first-match-index table per (b,s) row is
   built with compares + max-reduce; gathered through one-hot matmuls.
 - pipeline: per-batch GEMM stream with next batch's transposes interleaved;
   2-bank PSUM drain tiles; drains split ScalarE/VectorE.
"""

import math

import numpy as np

import concourse.mybir as mybir
import concourse.tile as tile
from concourse import bacc
from concourse.bass_utils import run_bass_kernel_spmd
from concourse.masks import make_identity

# problem shape (hardcoded per harness contract)
B, S, W, H = 32, 16, 128, 256
P = S * (S - 1) // 2  # 120
N_CORES = 8
BL = B // N_CORES  # 4 batches per core
ROWS = BL * P * W  # 61440 output rows per core
NSH = 2  # h halves of 128
R = BL * S  # 64 target rows per core
SC = 4  # s-chunking of loads/transposes

FP = mybir.dt.float32
BF = mybir.dt.bfloat16
I32 = mybir.dt.int32
ALU = mybir.AluOpType
AF = mybir.ActivationFunctionType


def _pbase(i):
    # first pair index with first element i in row-major triu order
    return sum(S - 1 - k for k in range(i))


def build_nc():
    nc = bacc.Bacc("TRN2", target_bir_lowering=False, debug=False)

    x_d = nc.dram_tensor("x", [BL, S, W, H], FP, kind="ExternalInput")
    t_d = nc.dram_tensor("target", [BL, S, W], I32, kind="ExternalInput")
    sc_d = nc.dram_tensor("scale", [1, 1], FP, kind="ExternalInput")
    scores_d = nc.dram_tensor("scores", [ROWS, W], BF, kind="ExternalOutput")
    tgt_d = nc.dram_tensor("tgt", [ROWS], I32, kind="ExternalOutput")

    with tile.TileContext(nc) as tc:
        with (
            tc.tile_pool(name="consts", bufs=1) as consts,
            tc.tile_pool(name="xraw", bufs=BL) as xraw_pool,
            tc.tile_pool(name="xt", bufs=BL * NSH) as xt_pool,
            tc.tile_pool(name="outs", bufs=6) as out_pool,
            tc.tile_pool(name="small", bufs=2) as small,
            tc.tile_pool(name="intp", bufs=2) as intp,
            tc.tile_pool(name="cmps", bufs=16) as cmps,
            tc.tile_pool(name="ps_mm", bufs=2, space="PSUM") as ps_mm,
            tc.tile_pool(name="ps_int", bufs=2, space="PSUM") as ps_int,
            tc.tile_pool(name="ps_tp", bufs=2, space="PSUM") as ps_tp,
        ):
            # ---------------- constants ----------------
            ident = consts.tile([128, 128], FP)
            make_identity(nc, ident[:])
            ident_bf = consts.tile([128, 128], BF)
            make_identity(nc, ident_bf[:])
            # wv[r, w] = W - w  (weight that makes max-reduce pick first match)
            wv = consts.tile([R, W], FP)
            nc.gpsimd.iota(
                wv[:], pattern=[[-1, W]], base=W, channel_multiplier=0,
                allow_small_or_imprecise_dtypes=True,
            )
            ones_row = consts.tile([1, 128], FP)
            nc.gpsimd.memset(ones_row[:], 1.0)

            # ---------------- PE warm-up (HAM clock gate) ----------------
            warm_in = consts.tile([128, 128], BF)
            nc.vector.memset(warm_in[:], 1.0)
            warm_ps = ps_mm.tile([128, 1024], FP, tag="ps_mm")
            NWARM = 90
            for k in range(NWARM):
                nc.tensor.matmul(
                    warm_ps[:, 0:128], warm_in[:], warm_in[:],
                    start=(k == 0), stop=(k == NWARM - 1),
                )

            # ---------------- input loads ----------------
            xraw = [[None] * (S // SC) for _ in range(BL)]
            for b in range(BL):
                qorder = reversed(range(S // SC)) if b == 0 else range(S // SC)
                for q in qorder:
                    xr = xraw_pool.tile([W, SC, H], FP, tag="xraw")
                    nc.sync.dma_start(
                        xr[:],
                        x_d[b, q * SC : (q + 1) * SC].rearrange("s w h -> w s h"),
                    )
                    xraw[b][q] = xr

            tmat_i = small.tile([R, W], I32)
            nc.sync.dma_start(tmat_i[:], t_d.ap().rearrange("b s w -> (b s) w"))
            tmat_f = small.tile([R, W], FP)
            nc.vector.tensor_copy(tmat_f[:], tmat_i[:])

            # host-computed norm scale -> broadcast to [128, 1]
            scale_sb = small.tile([1, 1], FP)
            nc.gpsimd.dma_start(scale_sb[:], sc_d[:, :])
            ps_bc = ps_int.tile([128, 1], FP, tag="ps_int")
            nc.tensor.matmul(
                ps_bc[:], ones_row[:], scale_sb[:], start=True, stop=True
            )
            scale128 = consts.tile([128, 1], FP)
            nc.vector.tensor_copy(scale128[:], ps_bc[:])

            # ---------------- int path: analysis (cheap, early) ----------
            # first-index table fi[(b,s), c] = min{w: target[b,s,w]==c} else 0
            maxv = intp.tile([R, 16], FP, tag="maxv")
            for c in range(16):
                eq = intp.tile([R, W], FP, tag="eq")
                nc.vector.tensor_scalar(
                    eq[:], tmat_f[:], float(c), None, op0=ALU.is_equal
                )
                val = intp.tile([R, W], FP, tag="val")
                nc.vector.tensor_tensor(val[:], eq[:], wv[:], op=ALU.mult)
                nc.vector.reduce_max(
                    maxv[:, c : c + 1], val[:], axis=mybir.AxisListType.X
                )
            gt0 = intp.tile([R, 16], FP, tag="gt0")
            nc.vector.tensor_scalar(gt0[:], maxv[:], 0.0, None, op0=ALU.is_gt)
            wmin = intp.tile([R, 16], FP, tag="wmin")
            nc.vector.tensor_scalar(
                wmin[:], maxv[:], -1.0, float(W), op0=ALU.mult, op1=ALU.add
            )
            fi = intp.tile([R, 16], FP, tag="fi")
            nc.vector.tensor_tensor(fi[:], gt0[:], wmin[:], op=ALU.mult)
            ps_fi = ps_int.tile([16, R], FP, tag="ps_int")
            nc.tensor.transpose(ps_fi[:], fi[:], ident[0:R, 0:R])
            fiT = intp.tile([16, R], BF, tag="fiT")
            nc.vector.tensor_copy(fiT[:], ps_fi[:])

            # tmatT[w, r] then one-hot OT[w, r, c] = (tmatT[w,r] == c)
            ps_tm = ps_int.tile([128, R], FP, tag="ps_int")
            nc.tensor.transpose(ps_tm[:], tmat_f[:], ident[0:R, 0:R])
            tmatT = intp.tile([128, R], FP, tag="tmatT")
            nc.vector.tensor_copy(tmatT[:], ps_tm[:])
            OT = intp.tile([128, R, 16], BF, tag="OT")
            for c in range(16):
                nc.vector.tensor_scalar(
                    OT[:, :, c], tmatT[:], float(c), None, op0=ALU.is_equal
                )

            # ---------------- pipeline building blocks ----------------
            xt = []
            for _b in range(BL):
                row = []
                for _h in range(NSH):
                    xt_t = xt_pool.tile([128, S * W], BF, tag="xt")
                    row.append(xt_t)
                xt.append(row)

            xs_cache = {}

            def emit_xs_unit(b, q, hf):
                key = (b, q)
                if key not in xs_cache:
                    xs = xraw_pool.tile([W, SC, H], BF, tag="xs")
                    nc.vector.tensor_scalar(
                        xs[:], xraw[b][q][:], scale128[:], None, op0=ALU.mult
                    )
                    xs_cache[key] = xs
                xs = xs_cache[key]
                ps_t = ps_tp.tile([128, SC * 128], BF, tag="ps_tp")
                for sl in range(SC):
                    nc.tensor.transpose(
                        ps_t[:, sl * 128 : (sl + 1) * 128],
                        xs[:, sl, hf * 128 : (hf + 1) * 128],
                        ident_bf[:],
                    )
                s0 = q * SC
                nc.vector.tensor_copy(
                    xt[b][hf][:, s0 * W : (s0 + SC) * W], ps_t[:]
                )

            def emit_xs_chunk(b, q):
                for hf in range(NSH):
                    emit_xs_unit(b, q, hf)

            drains = [0]

            def emit_gemm(b, i):
                j0 = i + 1
                L = S - 1 - i
                stage = out_pool.tile([128, L * W], BF, tag="outs")
                cols = L * W
                tiles = []  # (psum_tile, col0, ncols)
                c = 0
                while c < cols:
                    n = min(1024, cols - c)
                    pt = ps_mm.tile([128, 1024], FP, tag="ps_mm")
                    tiles.append((pt, c, n))
                    c += n
                for hf in range(NSH):
                    lhs = xt[b][hf][:, i * W : (i + 1) * W]
                    for pt, c0, n in tiles:
                        for g in range(0, n, 512):
                            m = min(512, n - g)
                            nc.tensor.matmul(
                                pt[:, g : g + m],
                                lhs,
                                xt[b][hf][
                                    :, j0 * W + c0 + g : j0 * W + c0 + g + m
                                ],
                                start=(hf == 0), stop=(hf == 1),
                            )
                dk = drains[0]
                for ti, (pt, c0, n) in enumerate(tiles):
                    dst = stage[:, c0 : c0 + n]
                    # two tiles of a group drain on different engines so the
                    # PSUM banks recycle in parallel; singles alternate
                    use_act = (ti == 0) if len(tiles) > 1 else (dk % 2 == 0)
                    if use_act:
                        nc.scalar.copy(dst, pt[:, :n])
                    else:
                        nc.vector.tensor_copy(dst, pt[:, :n])
                    dk += 1
                drains[0] = dk
                r0 = b * P + _pbase(i)
                nc.sync.dma_start(
                    scores_d.ap()[r0 * W : (r0 + L) * W, :].rearrange(
                        "(pp w) v -> w pp v", w=W
                    ),
                    stage[:].rearrange("w (pp v) -> w pp v", v=W),
                )

            cmp_sb = []
            tgt2d = tgt_d.ap().rearrange("(r v) -> r v", v=W)

            def emit_cmp_bank(kb):
                ps_cmp = ps_tp.tile([16, 512], BF, tag="ps_tp")
                for k in range(4):
                    rr = kb * 4 + k
                    nc.tensor.transpose(
                        ps_cmp[:, k * 128 : (k + 1) * 128],
                        OT[:, rr, :],
                        ident_bf[:],
                    )
                cs = cmps.tile([16, 512], BF, tag="cmp_sb")
                nc.vector.tensor_copy(cs[:], ps_cmp[:])
                cmp_sb.append(cs)

            def emit_tgt_group(bb, tt):
                # final matmul tgt[j, (r,w)] = fi[bb][t0[r,w], j]
                ps_tg = ps_int.tile([16, 512], FP, tag="ps_int")
                nc.tensor.matmul(
                    ps_tg[:],
                    fiT[:, bb * S : (bb + 1) * S],
                    cmp_sb[bb * 4 + tt][:],
                    start=True, stop=True,
                )
                tg_i = intp.tile([16, 512], I32, tag="tg_i")
                nc.vector.tensor_copy(tg_i[:], ps_tg[:])
                for k in range(4):
                    i = tt * 4 + k
                    if i >= S - 1:
                        continue
                    L = S - 1 - i
                    r0 = bb * P + _pbase(i)
                    nc.gpsimd.dma_start(
                        tgt2d[r0 : r0 + L, :],
                        tg_i[i + 1 : S, k * 128 : (k + 1) * 128],
                    )

            # ---------------- per-batch pipeline ----------------
            for q in reversed(range(S // SC)):
                emit_xs_chunk(0, q)
            for b in range(BL):
                i_order = (
                    list(reversed(range(S - 1))) if b == 0 else list(range(S - 1))
                )
                nxt = (
                    [(b + 1, q, hf) for q in range(S // SC) for hf in range(NSH)]
                    if b + 1 < BL
                    else []
                )
                for idx, i in enumerate(i_order):
                    emit_gemm(b, i)
                    if nxt and idx % 2 == 1:
                        emit_xs_unit(*nxt.pop(0))
                for u in nxt:
                    emit_xs_unit(*u)
                if b == 0:
                    for kb in range(R // 4):
                        emit_cmp_bank(kb)
                    for bb in range(BL):
                        for tt in range(4):
                            emit_tgt_group(bb, tt)

    nc.compile()
    return nc


_NC = None


def _get_nc():
    global _NC
    if _NC is None:
        _NC = build_nc()
    return _NC


def kernel(x: np.ndarray, target: np.ndarray):
    x = np.ascontiguousarray(x, dtype=np.float32)
    target = np.ascontiguousarray(target, dtype=np.int32)

    scale = np.float32(
        math.sqrt(5.0)
        / math.sqrt(float(np.sum(np.square(x, dtype=np.float64))))
    )
    sc = np.array([[scale]], dtype=np.float32)

    in_maps = []
    for c in range(N_CORES):
        in_maps.append(
            {
                "x": np.ascontiguousarray(x[c * BL : (c + 1) * BL]),
                "target": np.ascontiguousarray(target[c * BL : (c + 1) * BL]),
                "scale": sc,
            }
        )

    nc = _get_nc()
    res = run_bass_kernel_spmd(nc, in_maps, core_ids=list(range(N_CORES)))
    scores = np.concatenate(
        [np.asarray(r["scores"]) for r in res.results], axis=0
    ).astype(np.float32)
    tgt = np.concatenate([r["tgt"] for r in res.results], axis=0)
    return scores, tgt
